# revision 1
# baseline (speedup 1.0000x reference)
"""i0e(z) (exponentially scaled modified Bessel I0) on 8 TRN2 NeuronCores.

Math: with t = 1/sqrt(1+2*pi*x), u = t^2,
    i0e(x) ~= min(A(x), B(u)*t)
A: deg-9 poly in x, minimax-fit on [0, 3.75] with a pinned +g*x^9 term so the
   extrapolation beyond 3.75 stays above the reference (min() then never
   picks it there); B: cubic in u fit on [3.75, 100], whose natural
   extrapolation stays above the reference on [0, 3.75).

Per 128x4096 tile: DMA in -> ACT Sqrt(2*pi*x+1) -> DVE recip_approx_fast
-> 3 fused custom-DVE Horner insts (A) + 1 (B, incl. *t) -> tensor_tensor
min -> DMA out.  Data-parallel: rows sharded 8 ways, no communication.
"""
import numpy as np

P = 128
ROWS, COLS = 16384, 4096
NCORES = 8
SHARD = ROWS // NCORES  # 2048
RT = SHARD // P         # 16 row tiles per core
CT = 2          # col tiles
W = COLS // CT  # 2048 free-dim per tile
TWO_PI = 6.283185307179586

# A(x) = sum CA[k] x^k (deg 9, incl. pinned 3e-8 x^9); B(u) = sum CB[k] u^k.
CA = [0.9999880254525719, -0.9994343752665157, 0.745597234577468,
      -0.40331336400719653, 0.1615124409019363, -0.046668339804624266,
      0.009084116253358684, -0.001050432570739364, 5.375429555518828e-05,
      3e-08]
CB = [1.0000398601852005, 1.1546096589339048, 9.820096064171423, 0.0]

_NC_CACHE = {}


def _register_ops():
    """Define the three fused Horner ops and register them in dve_ops.OPS
    (runtime registration; sha pinned from lower() like DveOp.compile)."""
    import concourse.dve_ops as dve_ops
    from concourse.dve_ops import DveOp, OPS
    from concourse.dve_spec import (
        Spec, Src0, Src1, C0, C1, C2, C3, sq, lower, _spill_c3_to_src1,
        _has_src1,
    )
    from concourse.dve_uop import DveOpSpec

    if "I0E_ACC4" in dve_ops._SUB_OPCODE_FOR_NAME:
        return (dve_ops.OPS[dve_ops._SUB_OPCODE_FOR_NAME["I0E_ACC4"] - 1],
                dve_ops.OPS[dve_ops._SUB_OPCODE_FOR_NAME["I0E_STEP3"] - 1],
                dve_ops.OPS[dve_ops._SUB_OPCODE_FOR_NAME["I0E_TAIL"] - 1])

    def mk(name, body, ref):
        shas = {}
        for ver in ("v3", "v4"):
            s = DveOpSpec(name=name, opcode=1, uops=lower(body_spec(body, ref), ver=ver),
                          rd1_en=_has_src1(body_spec(body, ref)))
            shas[ver] = s.sha(ver)
        op = DveOp(name, body_spec(body, ref), subdim=False, uops_sha=shas)
        OPS.append(op)
        row = dve_ops._CUSTOM_DVE_ROW_BASE + len(OPS) - 1
        dve_ops._SUB_OPCODE_FOR_NAME[name] = row
        dve_ops.CUSTOM_DVE_SPECS[name] = op.spec
        return op

    def body_spec(body, ref):
        return Spec(body=body, reference=ref)

    # acc = ((C0*x + C1)*x + C2)*x + C3   (C3 latched via [P,1] in1)
    acc4 = mk(
        "I0E_ACC4",
        _spill_c3_to_src1(((C0 * Src0 + C1) * Src0 + C2) * Src0 + C3),
        lambda in0, in1, s0, s1, imm2:
            (((s0 * in0 + s1) * in0 + imm2) * in0
             + in1.reshape(in1.shape[0], -1)[:, :1]).astype(np.float32),
    )
    # acc = ((acc*x + C0)*x + C1)*x + C2
    step3 = mk(
        "I0E_STEP3",
        ((Src0 * Src1 + C0) * Src1 + C1) * Src1 + C2,
        lambda in0, in1, s0, s1, imm2:
            (((in0 * in1 + s0) * in1 + s1) * in1 + imm2).astype(np.float32),
    )
    # fhi = (((C0*u + C1)*u + C2)*u + C3)*t, u = t^2  (C3 latched via in1)
    _u = sq(Src0)
    tail = mk(
        "I0E_TAIL",
        _spill_c3_to_src1((((C0 * _u + C1) * _u + C2) * _u + C3) * Src0),
        lambda in0, in1, s0, s1, imm2:
            ((((s0 * in0 * in0 + s1) * in0 * in0 + imm2) * in0 * in0
              + in1.reshape(in1.shape[0], -1)[:, :1]) * in0).astype(np.float32),
    )
    return acc4, step3, tail


def _build():
    import concourse.bacc as bacc
    import concourse.tile as tile
    import concourse.mybir as mybir
    from contextlib import ExitStack

    acc4, step3, tail = _register_ops()
    f32 = mybir.dt.float32
    nc = bacc.Bacc("TRN2", debug=False)
    x_d = nc.dram_tensor("x", [SHARD, COLS], f32, kind="ExternalInput")
    o_d = nc.dram_tensor("o", [SHARD, COLS], f32, kind="ExternalOutput")

    with tile.TileContext(nc) as tc, ExitStack() as ctx:
        cpool = ctx.enter_context(tc.tile_pool(name="consts", bufs=1))
        c_a = cpool.tile([P, 1], f32)
        nc.vector.memset(c_a[:], CA[6])
        c_b = cpool.tile([P, 1], f32)
        nc.vector.memset(c_b[:], CB[0])
        xp = ctx.enter_context(tc.tile_pool(name="x", bufs=3))
        sp = ctx.enter_context(tc.tile_pool(name="s", bufs=2))
        tp = ctx.enter_context(tc.tile_pool(name="t", bufs=2))
        a1p = ctx.enter_context(tc.tile_pool(name="a1", bufs=2))
        a2p = ctx.enter_context(tc.tile_pool(name="a2", bufs=2))
        fhp = ctx.enter_context(tc.tile_pool(name="fh", bufs=2))
        outp = ctx.enter_context(tc.tile_pool(name="out", bufs=3))
        for r in range(RT):
          for c in range(CT):
              xt = xp.tile([P, W], f32)
              nc.sync.dma_start(xt[:], x_d[r * P:(r + 1) * P, c * W:(c + 1) * W])
              st = sp.tile([P, W], f32)
              nc.scalar.activation(st[:], xt[:],
                                   mybir.ActivationFunctionType.Sqrt,
                                   bias=1.0, scale=TWO_PI)
              tt = tp.tile([P, W], f32)
              nc.vector.reciprocal_approx_fast(tt[:], st[:])
              a1 = a1p.tile([P, W], f32)
              nc.vector._custom_dve(acc4, out=a1[:], in0=xt[:], in1=c_a[:],
                                    s0=CA[9], s1=CA[8], imm2=CA[7])
              a2 = a2p.tile([P, W], f32)
              nc.vector._custom_dve(step3, out=a2[:], in0=a1[:], in1=xt[:],
                                    s0=CA[5], s1=CA[4], imm2=CA[3])
              a3 = a1p.tile([P, W], f32)
              nc.vector._custom_dve(step3, out=a3[:], in0=a2[:], in1=xt[:],
                                    s0=CA[2], s1=CA[1], imm2=CA[0])
              fh = fhp.tile([P, W], f32)
              nc.vector._custom_dve(tail, out=fh[:], in0=tt[:], in1=c_b[:],
                                    s0=CB[3], s1=CB[2], imm2=CB[1])
              ot = outp.tile([P, W], f32)
              nc.vector.tensor_tensor(ot[:], a3[:], fh[:],
                                      op=mybir.AluOpType.min)
              nc.sync.dma_start(o_d[r * P:(r + 1) * P, c * W:(c + 1) * W], ot[:])
    nc.compile()
    return nc


def _get_nc():
    if "nc" not in _NC_CACHE:
        _NC_CACHE["nc"] = _build()
    return _NC_CACHE["nc"]


def kernel(z: np.ndarray) -> np.ndarray:
    from concourse import bass_utils
    nc = _get_nc()
    z = np.ascontiguousarray(z, dtype=np.float32)
    assert z.shape == (ROWS, COLS), z.shape
    in_maps = [{"x": z[i * SHARD:(i + 1) * SHARD]} for i in range(NCORES)]
    res = bass_utils.run_bass_kernel_spmd(nc, in_maps,
                                          core_ids=list(range(NCORES)))
    return np.concatenate([r["o"] for r in res.results], axis=0)

